# revision 32
# baseline (speedup 1.0000x reference)
"""TRN2 Bass kernel for 2-layer GAT + linear head (nn_GAT_15659450761218).

Strategy (8 NeuronCores, node-sharded by destination):
  - Algebraic collapse: attention logits are linear functionals of node
    features (as1 = x @ (W1_h a_src1_h)); layer-1 aggregation happens in
    input space (12 wide); layer 2 + head collapse to a [36,9] projection
    (z, as2, ad2), so the second message passing is only 6 wide.
  - Softmax division is deferred past the segment sums (denominator is
    per-(dst, head) constant), and the segment max is skipped (logits are
    O(1); exact in exact arithmetic, fp16-safe — validated vs reference).
  - Host routes edges: per core, dsts sorted by degree, padded into
    per-degree-bucket slots; slot layout [128 partitions, L slots] is
    dst-major so segment sums are innermost-axis ops and the dst-side
    logit is a broadcast along slots. Gathered (halo) source features are
    shipped pre-routed; the program is compiled per input.
  - Two launches: A computes layer-1 + the 9-wide projection per shard;
    host exchanges the halo; B computes layer-2 + head.
  - All hot-path data is fp16: the DVE runs 2x on packed 16-bit operands,
    matmuls are single-pass fp16 (PSUM fp32), and segment sums use
    pairwise halving adds (fp16, 2x) + a short fp32-out reduce.
  - The fin projection transposes bn via PE (identity matmul) instead of
    element-granular DMA, then runs a [36,9] matmul with per-partition
    bias applied during PSUM evacuation.

Canonical enumerations:
  sorted dst position j = t*128 + p   (tile-major; tiles sorted by degree)
  device node layout    (p, t)        (partition p, free index t)
  slot layout           [128, L], per-partition slot l = offs[t] + k
  zsd/out column        c = t*128 + p = j
"""

import os
import sys
import types

sys.path.insert(0, "/opt/trn_rl_repo")

import numpy as np

# NTFF profile hook for timing runs (missing antenv.axon_hooks in image).
if "antenv.axon_hooks" not in sys.modules:
    try:
        from trn_agent_boot.trn_boot import _ntff_profile_via_ctypes

        _mod = types.ModuleType("antenv.axon_hooks")
        _hook = _ntff_profile_via_ctypes("/opt/axon/libaxon_pjrt.so")
        _mod.get_axon_ntff_profile_hook = lambda: _hook
        _mod.set_axon_ntff_profile_hook = lambda h: None
        sys.modules["antenv.axon_hooks"] = _mod
    except Exception:
        pass

from concourse import bacc, tile, mybir  # noqa: E402
from concourse.bass_utils import run_bass_kernel_spmd as _run_spmd  # noqa: E402


def run_bass_kernel_spmd(nc, in_maps, core_ids, trace=False):
    """One retry on transient device errors (rare NRT exec-unit flake)."""
    try:
        return _run_spmd(nc, in_maps, core_ids, trace=trace)
    except Exception:
        import time as _time
        _time.sleep(2.0)
        return _run_spmd(nc, in_maps, core_ids, trace=trace)

F32 = mybir.dt.float32
F16 = mybir.dt.float16
AX = mybir.AxisListType
ALU = mybir.AluOpType
ACTF = mybir.ActivationFunctionType

N = 20000
E = 320000
H_IN = 12
C = 128
HEADS = 3
NCORES = 8
NSH = N // NCORES       # 2500
NPAD = 2560             # 20 tiles of 128 sorted dsts per core
NTILES = NPAD // 128
NEG = -30000.0          # pad-slot logit (exp underflows to exactly 0)
G = 8                   # fold factor for the slot matmul (8 x 16 features)
NF = 16                 # padded feature count (12 x + mask + 3 zero)
GRP = 1024              # streaming group width for the as1 matmul

TRACE = bool(os.environ.get("GAT_TRACE"))
LAST_TIMES = {}

# j = t*128 + p  <->  flat (p, t) index p*NTILES + t
_PT2J = (np.arange(NTILES)[None, :] * 128
         + np.arange(128)[:, None]).reshape(NPAD)


def _f16(x):
    return np.ascontiguousarray(np.asarray(x, dtype=np.float16))


# ----------------------------------------------------------------------------
# host-side routing
# ----------------------------------------------------------------------------

def _route(edge_index):
    """Per-core slot routing. Returns shared Kt plus per-core structures."""
    src = np.concatenate([edge_index[0], np.arange(N, dtype=np.int64)])
    dst = np.concatenate([edge_index[1], np.arange(N, dtype=np.int64)])

    cores = []
    kt_all = []
    for c in range(NCORES):
        lo, hi = c * NSH, (c + 1) * NSH
        m = (dst >= lo) & (dst < hi)
        csrc, cdst = src[m], (dst[m] - lo)
        deg = np.bincount(cdst, minlength=NPAD)
        order = np.argsort(deg, kind="stable")  # sorted pos j -> local dst id
        eo = np.argsort(cdst, kind="stable")
        es, ed = csrc[eo], cdst[eo]
        starts = np.searchsorted(ed, np.arange(NPAD))
        ends = np.searchsorted(ed, np.arange(NPAD) + 1)
        kt = np.zeros(NTILES, dtype=np.int64)
        for t in range(NTILES):
            mx = deg[order[128 * t:128 * t + 128]].max()
            kt[t] = max(8, int(np.ceil(mx / 8)) * 8)
        kt_all.append(kt)
        cores.append(dict(order=order, es=es, starts=starts, ends=ends, lo=lo))

    kt_uni = np.max(np.stack(kt_all), axis=0)
    assert kt_uni.max() <= 128, f"degree bucket overflow: {kt_uni}"
    offs = np.concatenate([[0], np.cumsum(kt_uni)]).astype(np.int64)
    L = int(offs[-1])

    for core in cores:
        order, es, starts, ends = (core["order"], core["es"],
                                   core["starts"], core["ends"])
        slot_src = np.full((128, L), -1, dtype=np.int64)
        for j in range(NPAD):
            t, p = divmod(j, 128)
            orig = order[j]
            s0, s1 = starts[orig], ends[orig]
            k = s1 - s0
            slot_src[p, offs[t]:offs[t] + k] = es[s0:s1]
        core["slot_src"] = slot_src
        core["own_global"] = np.clip(core["order"] + core["lo"], 0, N - 1)
        core["own_valid"] = core["order"] < NSH
    regions = []
    t0 = 0
    for t in range(1, NTILES + 1):
        if t == NTILES or kt_uni[t] != kt_uni[t0]:
            regions.append((t0, t, int(kt_uni[t0])))
            t0 = t
    # kernel A splits the first region so its as1 pipeline fills fast
    regions_a = list(regions)
    if regions_a and regions_a[0][1] - regions_a[0][0] > 3:
        (t0, t1, K) = regions_a[0]
        regions_a[0:1] = [(t0, t0 + 2, K), (t0 + 2, t1, K)]
    return dict(cores=cores, kt=kt_uni, offs=offs, L=L, regions=regions,
                regions_a=regions_a)


def _per_node_pt(per_j):
    """[NPAD(, F)] indexed by sorted pos j -> (p,t)-flat enumeration."""
    return per_j[_PT2J]


def _to_folded(per_slot, regions, offs):
    """[128, L, NF] -> folded [128, 16L], region-major columns:
    col 16*o0 + p_lo*Lr + (l - o0) within region (o0, o1)."""
    _, L, nf = per_slot.shape
    assert nf == NF
    out = np.zeros((128, 16 * L), dtype=per_slot.dtype)
    for j in range(G):
        blk = per_slot[16 * j:16 * (j + 1)]        # [16(p_lo), L, NF]
        for (t0, t1, _K) in regions:
            o0, o1 = int(offs[t0]), int(offs[t1])
            sub = blk[:, o0:o1, :]                 # [16, Lr, NF]
            out[16 * j:16 * (j + 1), 16 * o0:16 * o1] = (
                sub.transpose(2, 0, 1).reshape(NF, 16 * (o1 - o0)))
    return out


def _to_folded_nodes(per_node_pt):
    """[NPAD, NF] in (p,t)-flat order -> [128, 16*NTILES]."""
    arr = per_node_pt.reshape(128, NTILES, NF)
    out = np.zeros((128, 16 * NTILES), dtype=per_node_pt.dtype)
    for j in range(G):
        blk = arr[16 * j:16 * (j + 1)]
        out[16 * j:16 * (j + 1)] = blk.transpose(2, 0, 1).reshape(NF, 16 * NTILES)
    return out


def _blockdiag(u):
    """u [16, m] -> blockdiag lhsT [128, 8*m], cols ordered (h, j).

    (h, j) row order in the matmul output makes the DRAM roundtrip's
    read-back a single 3-dim DMA: (16j+q) merges into one w-stride dim."""
    m = u.shape[1]
    out = np.zeros((128, G * m), dtype=u.dtype)
    for j in range(G):
        for h in range(m):
            out[16 * j:16 * (j + 1), h * G + j] = u[:, h]
    return out


# ----------------------------------------------------------------------------
# device program builders
# ----------------------------------------------------------------------------

def _sub3d(dram, j, rows, X):
    """[16, rows, X] view of dram [G*rows, 16*X] for substream j:
    element (p_lo, h, l) = dram[rows*j + h, p_lo*X + l]."""
    return dram.ap()[rows * j: rows * (j + 1), :].rearrange(
        "h (p l) -> p h l", p=16)


def _halving_plan(K):
    """Pairwise-add halvings while the half stays even (the second operand
    of an odd-width halving is 4-byte misaligned in fp16, which drops the
    DVE out of 2x mode), then a short reduce."""
    seq = []
    k = K
    while k % 2 == 0 and k > 2 and (k // 2) % 2 == 0:
        h = k // 2
        seq.append((k, h))
        k = h
    return seq, k


def _tree_reduce(nc, tile_ap, nrows, nt, K, acc_out):
    """Segment-sum tile_ap [128, nrows, nt*K] (fp16) -> acc_out fp32.
    In-place pairwise halvings at stride-K granularity + final reduce."""
    v = tile_ap.rearrange("p r (t k) -> p r t k", k=K)
    seq, kfin = _halving_plan(K)
    for (k, h) in seq:
        nc.vector.tensor_add(v[:, :, :, 0:h], v[:, :, :, 0:h],
                             v[:, :, :, h:k])
    nc.vector.tensor_reduce(acc_out, v[:, :, :, 0:kfin], AX.X, ALU.add)


def _merged_readback(dram, rows, X):
    """One-DMA 3-dim view of dram [rows*G, 16*X] (rows (h j), cols (q l))
    -> [(j q)=128, rows, X]: j and q merge (j stride 16X == q stride X * 16)."""
    return dram.ap().rearrange("(h j) (q l) -> (j q) h l", h=rows, q=16)


def _build_kernel_A(rt):
    L, regions, offs = rt["L"], rt["regions_a"], rt["offs"]
    nc = bacc.Bacc(None, target_bir_lowering=False)

    # wpk packs ua | ub | xo | ident into one fp16 load
    WCOLS = 24 + 24 + 16 * NTILES + 128
    xf = nc.declare_dram_parameter("xf", [128, 16 * L], F16, isOutput=False)
    wpk_d = nc.declare_dram_parameter("wpk", [128, WCOLS], F16, isOutput=False)
    # xdm is region-major contiguous: [128, 12*w] block per region
    xdm = nc.declare_dram_parameter("xdm", [128, H_IN * L], F16, isOutput=False)
    pa_d = nc.declare_dram_parameter("pa", [36, 9], F16, isOutput=False)
    bias_d = nc.declare_dram_parameter("bias9", [9, 1], F32, isOutput=False)

    zsd_d = nc.declare_dram_parameter("zsd", [9, NPAD], F16, isOutput=True)

    as1_drams = [
        nc.dram_tensor(f"as1_dram{ri}",
                       [G * HEADS, 16 * (int(offs[t1]) - int(offs[t0]))], F16)
        for ri, (t0, t1, _K) in enumerate(regions)]
    ad1_dram = nc.dram_tensor("ad1_dram", [G * HEADS, 16 * NTILES], F16)

    NR = HEADS * (H_IN + 1)  # 39 rows: 0:3 ex (denominator), 3:39 ex*x

    with tile.TileContext(nc) as tc:
        with (
            tc.tile_pool(name="main", bufs=1) as pool,
            tc.tile_pool(name="psum", bufs=2, space="PSUM") as psum_pool,
            tc.tile_pool(name="psum_fin", bufs=1, space="PSUM") as psum_fin,
            nc.allow_low_precision(reason="fp16 edge pipeline, fp32 finals"),
        ):
            with nc.named_scope("load"):
                wpk = pool.tile([128, WCOLS], F16)
                pat = pool.tile([36, 9], F16)
                biast = pool.tile([9, 1], F32)
                nc.sync.dma_start(wpk[:], wpk_d.ap()[:, :])
                nc.gpsimd.dma_start(pat[:], pa_d.ap()[:, :])
                nc.gpsimd.dma_start(biast[:], bias_d.ap()[:, :])
                uat = wpk[:, 0:24]
                ubt = wpk[:, 24:48]
                xot = wpk[:, 48:48 + 16 * NTILES]
                idt = wpk[:, 48 + 16 * NTILES:WCOLS]
                warm = pool.tile([1, 1], F16)
                nc.scalar.activation(warm[:], warm[:], ACTF.Exp)

            # per-region xf loads first, in GRP-sized pieces so the first
            # matmul starts as soon as the first piece lands
            xft = [None] * len(regions)
            for ri, (t0, t1, K) in enumerate(regions):
                o0, o1 = int(offs[t0]), int(offs[t1])
                ncols = 16 * (o1 - o0)
                xft[ri] = pool.tile([128, ncols], F16, tag=f"xf{ri}",
                                    name=f"xft{ri}")
                for g0 in range(0, ncols, GRP):
                    g1 = min(g0 + GRP, ncols)
                    nc.sync.dma_start(xft[ri][:, g0:g1],
                                      xf.ap()[:, 16 * o0 + g0:16 * o0 + g1])
            xrt = [None] * len(regions)
            for ri, (t0, t1, K) in enumerate(regions):
                o0, o1 = int(offs[t0]), int(offs[t1])
                w = o1 - o0
                xrt[ri] = pool.tile([128, H_IN, w], F16, tag=f"xdm{ri}",
                                    name=f"xrt{ri}")
                nc.gpsimd.dma_start(
                    xrt[ri][:].rearrange("p f l -> p (f l)"),
                    xdm.ap()[:, H_IN * o0:H_IN * o1])

            # ---- ad1 (own-node dst logits), matmul output layout [24, 320]
            with nc.named_scope("mm_ad1"):
                ps_ad1 = psum_pool.tile([G * HEADS, GRP], F32, tag="ps")
                nc.tensor.matmul(ps_ad1[:, 0:16 * NTILES], ubt, xot,
                                 start=True, stop=True)
                ad1f = pool.tile([G * HEADS, 16 * NTILES], F16)
                nc.scalar.activation(ad1f[:], ps_ad1[:, 0:16 * NTILES],
                                     ACTF.Copy)
                nc.scalar.dma_start(ad1_dram.ap()[:, :], ad1f[:])
            din = pool.tile([128, HEADS, NTILES], F16)
            nc.gpsimd.dma_start(din[:],
                                _merged_readback(ad1_dram, HEADS, NTILES))

            acc = pool.tile([128, NR, NTILES], F32)
            rec = pool.tile([128, HEADS, NTILES], F32)
            bn = pool.tile([128, NTILES, 64], F16)
            rhsT = pool.tile([36, NTILES, 128], F16)
            zsb = pool.tile([9, NPAD], F16)
            rfl = rhsT[:].rearrange("q t p -> q (t p)")
            tp_done = 0
            zc_done = 0
            for ri, (t0, t1, K) in enumerate(regions):
                o0, o1 = int(offs[t0]), int(offs[t1])
                w = o1 - o0
                nt = t1 - t0
                xr = xrt[ri]
                ncols = 16 * w
                asf = pool.tile([G * HEADS, ncols], F16, tag=f"asf{ri}")
                with nc.named_scope(f"mm_as1_{ri}"):
                    for gi in range((ncols + GRP - 1) // GRP):
                        g0, g1 = GRP * gi, min(GRP * (gi + 1), ncols)
                        gw = g1 - g0
                        ps = psum_pool.tile([G * HEADS, GRP], F32, tag="ps")
                        for c0 in range(0, gw, 512):
                            c1 = min(c0 + 512, gw)
                            nc.tensor.matmul(ps[:, c0:c1], uat,
                                             xft[ri][:, g0 + c0:g0 + c1],
                                             start=True, stop=True)
                        nc.scalar.activation(asf[:, g0:g1], ps[:, 0:gw],
                                             ACTF.Copy)
                    nc.scalar.dma_start(as1_drams[ri].ap()[:, :], asf[:])
                exin = pool.tile([128, HEADS, w], F16, tag=f"exin{ri}")
                with nc.named_scope(f"tr{ri}"):
                    nc.sync.dma_start(exin[:],
                                      _merged_readback(as1_drams[ri], HEADS, w))
                with nc.named_scope(f"edge{ri}"):
                    exx = pool.tile([128, NR, w], F16, tag=f"exx{ri}")
                    vin = exin[:].rearrange("p r (t k) -> p r t k", k=K)
                    nc.vector.tensor_add(
                        vin, vin,
                        din[:, :, t0:t1].unsqueeze(3).broadcast_to(
                            [128, HEADS, nt, K]))
                    ex = exx[:, 0:HEADS, :]
                    nc.scalar.activation(ex, exin[:], ACTF.Prelu, alpha=0.2)
                    nc.scalar.activation(ex, ex, ACTF.Exp)
                    nc.vector.tensor_mul(
                        exx[:, HEADS:NR, :].rearrange(
                            "p (h f) l -> p h f l", h=HEADS),
                        ex.unsqueeze(2).broadcast_to([128, HEADS, H_IN, w]),
                        xr[:].unsqueeze(1).broadcast_to([128, HEADS, H_IN, w]))
                    _tree_reduce(nc, exx[:], NR, nt, K, acc[:, :, t0:t1])
                # region's node stats are final: fold fin work in here so the
                # projection tail only has the last tiles left
                with nc.named_scope(f"fin{ri}"):
                    nc.vector.reciprocal(rec[:, :, t0:t1],
                                         acc[:, 0:HEADS, t0:t1])
                    tmp = pool.tile([128, HEADS * H_IN, nt], F16,
                                    tag=f"bntmp{ri}")
                    nc.vector.tensor_mul(
                        tmp[:].rearrange("p (h f) t -> p h f t", h=HEADS),
                        acc[:, HEADS:NR, t0:t1].rearrange(
                            "p (h f) t -> p h f t", h=HEADS),
                        rec[:, :, t0:t1].unsqueeze(2).broadcast_to(
                            [128, HEADS, H_IN, nt]))
                    nc.vector.tensor_copy(
                        bn[:, t0:t1, 0:HEADS * H_IN],
                        tmp[:].rearrange("p q t -> p t q"))
                    while 2 * (tp_done + 1) <= t1:
                        g = tp_done
                        psT = psum_fin.tile([128, 128], F16, tag=f"tp{g % 2}")
                        nc.tensor.transpose(
                            psT[:],
                            bn[:, 2 * g:2 * g + 2, :].rearrange(
                                "p t q -> p (t q)"),
                            idt)
                        nc.vector.tensor_copy(rhsT[:, 2 * g, :], psT[0:36, :])
                        nc.scalar.activation(rhsT[:, 2 * g + 1, :],
                                             psT[64:100, :], ACTF.Copy)
                        tp_done += 1
                    # projection chunks whose t-range (4 tiles each) is ready
                    while 4 * (zc_done + 1) <= 2 * tp_done:
                        i = zc_done
                        psz = psum_fin.tile([9, 512], F32, tag=f"pz{i % 2}",
                                            name=f"psz{i}")
                        nc.tensor.matmul(psz[:], pat[:],
                                         rfl[:, 512 * i:512 * (i + 1)],
                                         start=True, stop=True)
                        if i % 2 == 0:
                            nc.vector.tensor_scalar_add(
                                zsb[:, 512 * i:512 * (i + 1)], psz[:],
                                biast[:])
                        else:
                            nc.scalar.activation(
                                zsb[:, 512 * i:512 * (i + 1)], psz[:],
                                ACTF.Identity, bias=biast[:])
                        nc.sync.dma_start(
                            zsd_d.ap()[:, 512 * i:512 * (i + 1)],
                            zsb[:, 512 * i:512 * (i + 1)])
                        zc_done += 1
    nc.compile()
    return nc


def _build_kernel_B(rt, out_const):
    L, regions, offs = rt["L"], rt["regions"], rt["offs"]
    nc = bacc.Bacc(None, target_bir_lowering=False)

    # az is region-major contiguous: [128, 6*w] block per region
    az = nc.declare_dram_parameter("az", [128, 2 * HEADS * L], F16, isOutput=False)
    ad2 = nc.declare_dram_parameter("ad2", [128, HEADS, NTILES], F16, isOutput=False)
    out_d = nc.declare_dram_parameter("outb", [128, NTILES], F32, isOutput=True)

    with tile.TileContext(nc) as tc:
        with (
            tc.tile_pool(name="main", bufs=1) as pool,
            nc.allow_low_precision(reason="fp16 edge pipeline, fp32 finals"),
        ):
            warm = pool.tile([1, 1], F16)
            nc.scalar.activation(warm[:], warm[:], ACTF.Exp)
            din = pool.tile([128, HEADS, NTILES], F16)
            nc.sync.dma_start(din[:], ad2.ap()[:, :, :])
            acc = pool.tile([128, 2 * HEADS, NTILES], F32)
            for ri, (t0, t1, K) in enumerate(regions):
                o0, o1 = int(offs[t0]), int(offs[t1])
                w = o1 - o0
                nt = t1 - t0
                with nc.named_scope(f"r{ri}"):
                    azr = pool.tile([128, 2 * HEADS, w], F16, tag=f"az{ri}")
                    eng = [nc.sync, nc.scalar, nc.gpsimd][ri % 3]
                    eng.dma_start(azr[:].rearrange("p r l -> p (r l)"),
                                  az.ap()[:, 6 * o0:6 * o1])
                    exr = azr[:, 0:HEADS, :]
                    zsr = azr[:, HEADS:, :]
                    vex = exr.rearrange("p r (t k) -> p r t k", k=K)
                    nc.vector.tensor_add(
                        vex, vex,
                        din[:, :, t0:t1].unsqueeze(3).broadcast_to(
                            [128, HEADS, nt, K]))
                    nc.scalar.activation(exr, exr, ACTF.Prelu, alpha=0.2)
                    nc.scalar.activation(exr, exr, ACTF.Exp)
                    nc.vector.tensor_mul(zsr, exr, zsr)
                    _tree_reduce(nc, azr[:], 2 * HEADS, nt, K,
                                 acc[:, :, t0:t1])
            with nc.named_scope("fin"):
                rec = pool.tile([128, HEADS, NTILES], F32)
                wz = pool.tile([128, HEADS, NTILES], F32)
                out_t = pool.tile([128, NTILES], F32)
                nc.vector.reciprocal(rec[:], acc[:, 0:HEADS, :])
                nc.vector.tensor_mul(wz[:], acc[:, HEADS:2 * HEADS, :],
                                     rec[:])
                nc.vector.tensor_add(out_t[:], wz[:, 0, :], wz[:, 1, :])
                nc.vector.tensor_add(out_t[:], out_t[:], wz[:, 2, :])
                nc.scalar.add(out_t[:], out_t[:], float(out_const))
                nc.sync.dma_start(out_d.ap()[:, :], out_t[:])
    nc.compile()
    return nc


# ----------------------------------------------------------------------------
# main entry
# ----------------------------------------------------------------------------

def kernel(**inputs):
    x = np.asarray(inputs["x"], np.float32)
    ei = np.asarray(inputs["edge_index"], np.int64)
    W1 = np.asarray(inputs["W1"], np.float32)
    a_src1 = np.asarray(inputs["a_src1"], np.float32)
    a_dst1 = np.asarray(inputs["a_dst1"], np.float32)
    b1 = np.asarray(inputs["b1"], np.float32)
    W2 = np.asarray(inputs["W2"], np.float32)
    a_src2 = np.asarray(inputs["a_src2"], np.float32)
    a_dst2 = np.asarray(inputs["a_dst2"], np.float32)
    b2 = np.asarray(inputs["b2"], np.float32)
    Wl = np.asarray(inputs["Wl"], np.float32)
    bl = np.asarray(inputs["bl"], np.float32)

    # ---- weight folds ----
    W1h = W1.reshape(H_IN, HEADS, C)
    Ua = np.stack([W1h[:, h, :] @ a_src1[h] for h in range(HEADS)], axis=1)
    Ub = np.stack([W1h[:, h, :] @ a_dst1[h] for h in range(HEADS)], axis=1)
    W2h = W2.reshape(HEADS * C, HEADS, C)
    Vz = np.stack([W2h[:, h, :] @ Wl[h * C:(h + 1) * C, 0] for h in range(HEADS)], axis=1)
    Vs = np.stack([W2h[:, h, :] @ a_src2[h] for h in range(HEADS)], axis=1)
    Vd = np.stack([W2h[:, h, :] @ a_dst2[h] for h in range(HEADS)], axis=1)
    V = np.concatenate([Vz, Vs, Vd], axis=1)          # [384, 9]
    P_all = np.concatenate(
        [W1h[:, h, :] @ V[h * C:(h + 1) * C, :] for h in range(HEADS)], axis=0)
    bias_row = b1 @ V                                  # [9]
    out_const = float(b2 @ Wl[:, 0] + bl[0])

    Ua16 = np.zeros((NF, HEADS), np.float32)
    Ua16[:H_IN] = Ua
    Ua16[H_IN] = NEG                                   # mask feature hook
    Ub16 = np.zeros((NF, HEADS), np.float32)
    Ub16[:H_IN] = Ub

    rt = _route(ei)
    L = rt["L"]

    ua = _f16(_blockdiag(Ua16))
    ub = _f16(_blockdiag(Ub16))
    pa = _f16(P_all)
    bias9 = np.ascontiguousarray(bias_row.reshape(9, 1), dtype=np.float32)
    ident = np.eye(128, dtype=np.float16)

    in_maps_a = []
    for c in range(NCORES):
        core = rt["cores"][c]
        ss = core["slot_src"]
        pad = ss < 0
        xs = np.where(pad[:, :, None], 0.0, x[np.clip(ss, 0, N - 1)])
        per_slot = np.concatenate([
            xs.astype(np.float32),
            pad[:, :, None].astype(np.float32),          # mask feature
            np.zeros((128, L, NF - H_IN - 1), np.float32),
        ], axis=2)
        xff = _f16(_to_folded(per_slot, rt["regions_a"], rt["offs"]))
        xdm = _f16(np.concatenate(
            [per_slot[:, int(rt["offs"][t0]):int(rt["offs"][t1]), 0:H_IN]
             .transpose(0, 2, 1).reshape(128, -1)
             for (t0, t1, _K) in rt["regions_a"]], axis=1))
        own = np.where(core["own_valid"][:, None], x[core["own_global"]], 0.0)
        own16 = np.concatenate(
            [own, np.zeros((NPAD, NF - H_IN), np.float32)], axis=1)
        xof = _f16(_to_folded_nodes(_per_node_pt(own16)))
        wpk = _f16(np.concatenate([ua, ub, xof, ident], axis=1))
        in_maps_a.append({
            "xf": xff, "wpk": wpk, "xdm": xdm,
            "pa": pa, "bias9": bias9,
        })

    nc_a = _build_kernel_A(rt)
    res_a = run_bass_kernel_spmd(nc_a, in_maps_a, list(range(NCORES)),
                                 trace=TRACE)
    if TRACE:
        LAST_TIMES["A"] = res_a.exec_time_ns
        LAST_TIMES["A_scopes"] = res_a.per_core_scope_times

    # zsd [9, NPAD] with col c = t*128 + p = sorted pos j
    zsd_full = np.zeros((N, 9), np.float32)
    for c in range(NCORES):
        zs = np.asarray(res_a.results[c]["zsd"], np.float32)
        core = rt["cores"][c]
        zs_local = np.zeros((NPAD, 9), np.float32)
        zs_local[core["order"]] = zs.T
        zsd_full[c * NSH:(c + 1) * NSH] = zs_local[:NSH]

    in_maps_b = []
    for c in range(NCORES):
        core = rt["cores"][c]
        ss = core["slot_src"]
        pad = ss < 0
        zse = np.where(pad[:, :, None], 0.0,
                       zsd_full[np.clip(ss, 0, N - 1), 0:3])
        ase = np.where(pad[:, :, None], NEG,
                       zsd_full[np.clip(ss, 0, N - 1), 3:6])
        ad2_j = np.where(core["own_valid"][:, None],
                         zsd_full[core["own_global"], 6:9], 0.0)
        ad2_pt = _per_node_pt(ad2_j).reshape(128, NTILES, 3)
        azf = np.concatenate(
            [ase.transpose(0, 2, 1), zse.transpose(0, 2, 1)], axis=1)
        in_maps_b.append({
            "az": _f16(np.concatenate(
                [azf[:, :, int(rt["offs"][t0]):int(rt["offs"][t1])]
                 .reshape(128, -1)
                 for (t0, t1, _K) in rt["regions"]], axis=1)),
            "ad2": _f16(ad2_pt.transpose(0, 2, 1)),
        })

    nc_b = _build_kernel_B(rt, out_const)
    res_b = run_bass_kernel_spmd(nc_b, in_maps_b, list(range(NCORES)),
                                 trace=TRACE)
    if TRACE:
        LAST_TIMES["B"] = res_b.exec_time_ns
        LAST_TIMES["B_scopes"] = res_b.per_core_scope_times

    out = np.zeros((N, 1), np.float32)
    for c in range(NCORES):
        ob = np.asarray(res_b.results[c]["outb"], np.float32)  # [128, NTILES]
        core = rt["cores"][c]
        o_local = np.zeros(NPAD, np.float32)
        o_local[core["order"]] = ob.T.reshape(NPAD)  # j = t*128+p -> ob[p, t]
        out[c * NSH:(c + 1) * NSH, 0] = o_local[:NSH]
    return out
